# revision 1
# baseline (speedup 1.0000x reference)
"""Sinkhorn optimal-transport transport-plan kernel for 8 Trainium2 NeuronCores.

Math (matches the reference):
    cost = max(sq_m[i] + sq_n[j] - 2 Hm@Hn^T, 0);  K = exp(-cost/eps)
    20x:  u <- mu / (K @ (nu / (K^T @ u)))
    v = nu / (K^T u);  P = diag(u) K diag(v)

Distribution: K is row-sharded, R = N/8 = 1024 rows per core.  Per Sinkhorn
iteration each core computes its partial of w = K^T u from its local rows
(PE matmuls, contracting over the partition axis), the length-N partial is
AllReduced (32 KB), then y = K x is computed locally from a transposed copy
KT streamed from HBM (so the PE can again contract over the partition axis).
Vectors live in SBUF in a partition-major [128, n/128] layout throughout, so
matmul weights/moving operands and the AllReduce bounce buffers never need
any transposes in the loop.

K is stored in fp16 (the K-rows copy stays resident in SBUF; KT in HBM):
elementwise quantization is ~5e-4 relative, and a numpy simulation of the
full pipeline shows ~6e-4 absmax-relative error in the final plan.  x and v
are ~1e-7 in magnitude, below the fp16 normal range, so they are carried
scaled by 2^20 (exact power of two) and the scale is divided back out of the
fp32 psum results / final plan.

kernel(H_m, H_n) takes the full inputs and returns the full (N, N) fp32 plan.
"""

import sys

for _p in ("/opt/trn_rl_repo", "/root/.axon_site", "/root/.axon_site/_ro/pypackages"):
    if _p not in sys.path:
        sys.path.append(_p)

import numpy as np

import concourse.bass as bass
import concourse.mybir as mybir
import concourse.tile as tile
from concourse.masks import make_identity

F32 = mybir.dt.float32
F16 = mybir.dt.float16
Exp = mybir.ActivationFunctionType.Exp

EPS = 0.05
ITERS = 20
SX = float(2**20)  # power-of-two scale keeping x, v in fp16 normal range

MAX_WAITS = 1  # walrus codegen allows only one attached sync wait per inst


def _split_excess_waits(nc, maxw=MAX_WAITS):
    """Walrus's per-instruction sync-wait slots are limited (a 4-wait Matmult
    fails codegen).  Tile's sem-assignment emits however many waits the
    vector clock requires, so split any excess onto same-engine NoOps
    inserted immediately before the instruction (engine queues execute in
    program order, so the semantics are identical)."""
    for bb in nc.main_func.blocks:
        new = []
        for ins in bb.instructions:
            si = ins.sync_info
            if si is not None and len(si.on_wait) > maxw:
                waits = list(si.on_wait)
                excess, keep = waits[:-maxw], waits[-maxw:]
                for i in range(0, len(excess), maxw):
                    nop = mybir.InstNoOp(
                        name=nc.get_next_instruction_name(),
                        engine=ins.engine,
                        bass_nofuse=True,
                        sync_info=mybir.SyncInfo(
                            on_wait=excess[i : i + maxw], on_update=[]
                        ),
                    )
                    new.append(nop)
                ins.sync_info = mybir.SyncInfo(
                    on_wait=keep, on_update=list(si.on_update)
                )
            new.append(ins)
        bb.instructions = new


def build_nc(N=8192, D=128, ncores=8, split_waits=True, iters=ITERS,
             collective=True, swizzle=True, tA=True, tB=True):
    assert D == 128 and N % (ncores * 128) == 0
    R = N // ncores  # local rows per core
    S = R // 128     # row stripes of 128
    C = N // 128     # column chunks of 128
    P = 128
    GW = min(1024, N)        # K-build psum granule width (2 psum banks)
    HNW = min(2048, N)       # resident width of hnT during the build phase
    KTG = 8                  # KT j-chunks per streamed granule in pass B

    nc = bass.Bass(num_devices=ncores)
    hmT = nc.declare_dram_parameter("hmT", [D, R], F32, isOutput=False)
    hnT = nc.declare_dram_parameter("hnT", [D, N], F32, isOutput=False)
    out = nc.declare_dram_parameter("out", [R, N], F32, isOutput=True)

    with tile.TileContext(nc) as tc:

        def bcast_from_pm(pool, dram, src_pm, M, name):
            """[128, M] f32 partition-major vec (src[p,c] = vec[c*128+p])
            -> [128, M*128] f16 tile (from `pool`) with the vector
            replicated on every partition in index order along free.
            Uses its own transient pools for scratch."""
            bc = pool.tile([P, M * P], F16, name=f"{name}_bc")
            tc.strict_bb_all_engine_barrier()
            with (
                tc.tile_pool(name=f"bc_sb_{name}", bufs=1) as bsb,
                tc.tile_pool(name=f"bc_ps_{name}", bufs=1, space="PSUM") as bps,
            ):
                tp_ps = bps.tile([M, P], F32, name=f"{name}_tp", tag="bc_tp")
                nc.tensor.transpose(tp_ps, src_pm, ident)
                row16 = bsb.tile([M, P], F16, name=f"{name}_row")
                nc.scalar.copy(row16, tp_ps)
                rdram = dram.tile([M, P], F16, name=f"{name}_dram")
                nc.sync.dma_start(out=rdram, in_=row16)
                rflat = rdram.rearrange("m p -> (m p)")[None, :]
                FL = min(2048, M * P)
                for f in range(0, M * P, FL):
                    flat = bsb.tile([1, FL], F16, name=f"{name}_flat{f}",
                                    tag="bc_flat", bufs=2)
                    nc.sync.dma_start(out=flat, in_=rflat[:, f : f + FL])
                    w = min(512, FL)
                    for t in range(0, FL, w):
                        mps = bps.tile([P, w], F32, name=f"{name}_mps{f}_{t}",
                                       tag="bc_mps", bufs=2)
                        nc.tensor.matmul(
                            out=mps, lhsT=ones_row16, rhs=flat[:, t : t + w],
                            start=True, stop=True,
                        )
                        nc.scalar.copy(bc[:, f + t : f + t + w], mps)
            return bc

        with (
            tc.tile_pool(name="persist", bufs=1) as sb,
            tc.tile_pool(name="dram", bufs=1, space="DRAM") as dram,
        ):
            # ---- persistent state ----
            k_sb = sb.tile([P, S * N], F16, name="k_sb")  # resident K rows
            NGR = C // KTG  # pass-B granules
            if swizzle:
                kt_dram = dram.tile([NGR, P, KTG * R], F16, name="kt_dram")
            else:
                kt_dram = dram.tile([N, R], F16, name="kt_dram")
            u_sb = sb.tile([P, S], F16, name="u_sb")
            nc.vector.memset(u_sb, 1.0)
            ident = sb.tile([P, P], F32, name="ident")
            make_identity(nc, ident)
            ones_col = sb.tile([P, 1], F32, name="ones_col")
            nc.vector.memset(ones_col, 1.0)
            ones_row16 = sb.tile([1, P], F16, name="ones_row16")
            nc.vector.memset(ones_row16, 1.0)
            hmT_sb = sb.tile([P, R], F32, name="hmT_sb")
            nc.sync.dma_start(out=hmT_sb, in_=hmT[:, :])
            bias_m = sb.tile([P, S], F32, name="bias_m")
            bias_n = sb.tile([P, C], F32, name="bias_n")
            v_pm = sb.tile([P, C], F32, name="v_pm")

            # ============ setup: biases, en/em broadcasts, K and KT ============
            with tc.tile_pool(name="setup_sb", bufs=2) as st:
                # bias_m[p,s] = -|Hm[s*128+p]|^2/eps ; bias_n[p,c] likewise.
                with tc.tile_pool(name="sq_ps", bufs=1, space="PSUM") as sqp:
                    ps_sqm = sqp.tile([P, S], F32, name="ps_sqm")
                    for q in range(0, R, 512):
                        qw = min(512, R - q)
                        sq_g = st.tile([P, qw], F32, name=f"hm2_{q}", tag="sq_g")
                        nc.vector.tensor_mul(
                            sq_g, hmT_sb[:, q : q + qw], hmT_sb[:, q : q + qw]
                        )
                        for k in range(qw // P):
                            s = (q + k * P) // P
                            nc.tensor.matmul(
                                out=ps_sqm[:, s : s + 1],
                                lhsT=sq_g[:, k * P : (k + 1) * P],
                                rhs=ones_col, start=True, stop=True,
                            )
                    nc.vector.tensor_scalar_mul(bias_m, ps_sqm, -1.0 / EPS)

                    ps_sqn = sqp.tile([P, C], F32, name="ps_sqn")
                    for h in range(0, N, HNW):
                        hn_h = st.tile([P, HNW], F32, name=f"hn_sq{h}", tag="hn_h")
                        nc.sync.dma_start(out=hn_h, in_=hnT[:, h : h + HNW])
                        for q in range(0, HNW, 512):
                            sq_g = st.tile([P, 512], F32, name=f"hn2_{h}_{q}",
                                           tag="sq_g")
                            nc.vector.tensor_mul(
                                sq_g, hn_h[:, q : q + 512], hn_h[:, q : q + 512]
                            )
                            for k in range(4):
                                c = (h + q) // P + k
                                nc.tensor.matmul(
                                    out=ps_sqn[:, c : c + 1],
                                    lhsT=sq_g[:, k * P : (k + 1) * P],
                                    rhs=ones_col, start=True, stop=True,
                                )
                    nc.vector.tensor_scalar_mul(bias_n, ps_sqn, -1.0 / EPS)

                    # en[j] = exp(-sq_n[j]/eps), em[i] = exp(-sq_m[i]/eps)
                    en_pm = st.tile([P, C], F32, name="en_pm", tag="e_pm", bufs=1)
                    nc.scalar.activation(en_pm, bias_n, Exp)
                    em_pm = st.tile([P, S], F32, name="em_pm", tag="e_pm2",
                                    bufs=1)
                    nc.scalar.activation(em_pm, bias_m, Exp)
                en_bc = bcast_from_pm(st, dram, en_pm, C, "en")  # [128, N] f16
                em_bc = bcast_from_pm(st, dram, em_pm, S, "em")  # [128, R] f16

                # K[i, j]  = exp(2/eps*G  - sq_m[i]/eps) * en[j]   (i on parts)
                # KT[j, i] = exp(2/eps*G^T - sq_n[j]/eps) * em[i]  (j on parts)
                with tc.tile_pool(name="build_ps", bufs=2, space="PSUM") as bp:
                    for h in range(0, N, HNW):
                        hn_h = st.tile([P, HNW], F32, name=f"hn_b{h}", tag="hn_h")
                        nc.sync.dma_start(out=hn_h, in_=hnT[:, h : h + HNW])
                        for s in range(S):
                            for g in range(0, HNW, GW):
                                gps = bp.tile([P, GW], F32, name=f"g{h}_{s}_{g}",
                                              tag="gps")
                                for q in range(0, GW, 512):
                                    nc.tensor.matmul(
                                        out=gps[:, q : q + 512],
                                        lhsT=hmT_sb[:, s * P : (s + 1) * P],
                                        rhs=hn_h[:, g + q : g + q + 512],
                                        start=True, stop=True,
                                    )
                                ksl = k_sb[:, s * N + h + g : s * N + h + g + GW]
                                nc.scalar.activation(
                                    ksl, gps, Exp, bias=bias_m[:, s : s + 1],
                                    scale=2.0 / EPS,
                                )
                                nc.vector.tensor_mul(
                                    ksl, ksl, en_bc[:, h + g : h + g + GW]
                                )
                        for jc in range(h // P, (h + HNW) // P):
                            g2 = bp.tile([P, R], F32, name=f"g2_{jc}", tag="g2")
                            for q in range(0, R, 512):
                                qw = min(512, R - q)
                                nc.tensor.matmul(
                                    out=g2[:, q : q + qw],
                                    lhsT=hn_h[:, jc * P - h : jc * P - h + P],
                                    rhs=hmT_sb[:, q : q + qw],
                                    start=True, stop=True,
                                )
                            kt_t = st.tile([P, R], F16, name=f"kt_b{jc}",
                                           tag="kt_b", bufs=3)
                            nc.scalar.activation(
                                kt_t, g2, Exp, bias=bias_n[:, jc : jc + 1],
                                scale=2.0 / EPS,
                            )
                            nc.vector.tensor_mul(kt_t, kt_t, em_bc)
                            if swizzle:
                                kt_out = kt_dram[
                                    jc // KTG, :,
                                    (jc % KTG) * R : (jc % KTG + 1) * R,
                                ]
                            else:
                                kt_out = kt_dram[jc * P : (jc + 1) * P, :]
                            nc.sync.dma_start(out=kt_out, in_=kt_t)

            # ======================= Sinkhorn loop =======================
            tc.strict_bb_all_engine_barrier()
            with (
                tc.tile_pool(name="loop_sb", bufs=2) as lp,
                tc.tile_pool(name="loop_ps", bufs=2, space="PSUM") as lpp,
            ):
                HC = C // 2  # AllReduce in two j-halves to hide its latency
                for it in range(iters + 1):
                    # pass A: w_partial = K_local^T u -> [128, C] part-major.
                    # The j-half AllReduce overlaps the other half's matmuls.
                    xh = []
                    for h in range(2):
                        psw = lpp.tile([P, HC], F32, name=f"psw{it}_{h}",
                                       tag=f"psw{h}")
                        if tA:
                            for c in range(HC):
                                cc = h * HC + c
                                for s in range(S):
                                    nc.tensor.matmul(
                                        out=psw[:, c : c + 1],
                                        lhsT=k_sb[
                                            :, s * N + cc * P
                                            : s * N + (cc + 1) * P
                                        ],
                                        rhs=u_sb[:, s : s + 1],
                                        start=(s == 0), stop=(s == S - 1),
                                    )
                        else:
                            nc.vector.memset(psw, 1000.0)
                        w_sb = lp.tile([P, HC], F32, name=f"w{it}_{h}",
                                       tag=f"w_sb{h}")
                        nc.scalar.copy(w_sb, psw)
                        w_in = dram.tile([P, HC], F32, name=f"w_in{it}_{h}",
                                         tag=f"w_in{h}", bufs=2)
                        w_out = dram.tile(
                            [P, HC], F32, name=f"w_out{it}_{h}", tag=f"w_out{h}",
                            bufs=2, addr_space="Shared",
                        )
                        nc.scalar.dma_start(out=w_in, in_=w_sb)
                        if collective:
                            nc.gpsimd.collective_compute(
                                "AllReduce", mybir.AluOpType.add,
                                replica_groups=[list(range(ncores))],
                                ins=[w_in.opt()], outs=[w_out.opt()],
                            )
                        else:  # single-core timeline modeling
                            nc.scalar.dma_start(out=w_out, in_=w_in)
                        wf_sb = lp.tile([P, HC], F32, name=f"wf{it}_{h}",
                                        tag=f"wf{h}")
                        nc.scalar.dma_start(out=wf_sb, in_=w_out)
                        rec = lp.tile([P, HC], F32, name=f"rec{it}_{h}",
                                      tag=f"rec{h}")
                        nc.vector.reciprocal(rec, wf_sb)
                        if it == iters:
                            # v' = SX*nu/w from the final w
                            nc.vector.tensor_scalar_mul(
                                v_pm[:, h * HC : (h + 1) * HC], rec, SX / N
                            )
                        else:
                            x_sb = lp.tile([P, HC], F16, name=f"x{it}_{h}",
                                           tag=f"x{h}")
                            nc.vector.tensor_scalar_mul(x_sb, rec, SX / N)
                            xh.append(x_sb)  # x' = SX*nu/w
                    if it == iters:
                        break

                    # pass B: y' = K_local x' via KT stream -> [128, S].
                    # PSUM allows only one open accumulation group per bank
                    # ("zero region"), so accumulate each granule's partial
                    # with per-column contiguous groups and sum in SBUF.
                    y_acc = lp.tile([P, S], F32, name=f"yacc{it}", tag="yacc")
                    if not tB:
                        nc.vector.memset(y_acc, 1.0)
                    ng = C // KTG
                    for g in range(ng if tB else 0):
                        kt_t = lp.tile(
                            [P, KTG * R], F16, name=f"kt{it}_{g}", tag="kt", bufs=3
                        )
                        if swizzle:
                            nc.sync.dma_start(out=kt_t, in_=kt_dram[g])
                        else:
                            nc.sync.dma_start(
                                out=kt_t.rearrange("p (jj i) -> p jj i", jj=KTG),
                                in_=kt_dram[
                                    g * KTG * P : (g + 1) * KTG * P, :
                                ].rearrange("(jj p) i -> p jj i", p=P),
                            )
                        psy = lpp.tile([P, S], F32, name=f"psy{it}_{g}", tag="psy")
                        for s in range(S):
                            for jj in range(KTG):
                                cc = g * KTG + jj
                                nc.tensor.matmul(
                                    out=psy[:, s : s + 1],
                                    lhsT=kt_t[
                                        :, jj * R + s * P : jj * R + (s + 1) * P
                                    ],
                                    rhs=xh[cc // HC][:, cc % HC : cc % HC + 1],
                                    start=(jj == 0), stop=(jj == KTG - 1),
                                )
                        if g == 0:
                            nc.vector.tensor_copy(y_acc, psy)
                        else:
                            nc.vector.tensor_add(y_acc, y_acc, psy)
                    rec2 = lp.tile([P, S], F32, name=f"rec2{it}", tag="rec2")
                    nc.vector.reciprocal(rec2, y_acc)
                    nc.vector.tensor_scalar_mul(u_sb, rec2, SX / N)  # mu*SX/y'

            # ==================== v and the transport plan ====================
            tc.strict_bb_all_engine_barrier()
            with tc.tile_pool(name="fin_sb", bufs=2) as fp:
                v_bc = bcast_from_pm(fp, dram, v_pm, C, "v")  # [128, N] f16
                u_div = fp.tile([P, S], F32, name="u_div", bufs=1)
                nc.vector.tensor_scalar_mul(u_div, u_sb, 1.0 / SX)

                FW = min(2048, N)
                for s in range(S):
                    for t in range(0, N, FW):
                        f16t = fp.tile([P, FW], F16, name=f"f16_{s}_{t}",
                                       tag="fin16")
                        nc.vector.tensor_mul(
                            f16t, k_sb[:, s * N + t : s * N + t + FW],
                            v_bc[:, t : t + FW],
                        )
                        f32t = fp.tile([P, FW], F32, name=f"f32_{s}_{t}",
                                       tag="fin32")
                        nc.vector.tensor_scalar_mul(f32t, f16t, u_div[:, s : s + 1])
                        nc.sync.dma_start(
                            out=out[s * P : (s + 1) * P, t : t + FW], in_=f32t
                        )
    if split_waits:
        _split_excess_waits(nc)
    return nc


_NC_CACHE = {}


def get_nc(N=8192, D=128, ncores=8):
    key = (N, D, ncores)
    if key not in _NC_CACHE:
        _NC_CACHE[key] = build_nc(N, D, ncores)
    return _NC_CACHE[key]


def make_in_maps(H_m, H_n, ncores=8):
    H_m = np.asarray(H_m, dtype=np.float32)
    H_n = np.asarray(H_n, dtype=np.float32)
    N = H_m.shape[0]
    R = N // ncores
    hnT = np.ascontiguousarray(H_n.T)
    return [
        {
            "hmT": np.ascontiguousarray(H_m[c * R : (c + 1) * R].T),
            "hnT": hnT,
        }
        for c in range(ncores)
    ]


def kernel(H_m, H_n):
    from concourse.bass_utils import run_bass_kernel_spmd

    ncores = 8
    nc = get_nc(N=np.asarray(H_m).shape[0], D=np.asarray(H_m).shape[1],
                ncores=ncores)
    in_maps = make_in_maps(H_m, H_n, ncores)
    res = run_bass_kernel_spmd(nc, in_maps, core_ids=list(range(ncores)))
    return np.concatenate([res.results[c]["out"] for c in range(ncores)], axis=0)

